# revision 14
# baseline (speedup 1.0000x reference)
"""Trainium2 Bass kernel for causal multi-head attention.

Problem: B=2, T=2048, D=1024, H=16 heads of dim 64, causal softmax,
fp32 weights, no qkv bias, output projection with bias.

Sharding (8 cores): core c handles batch b = c//4 and head group
g = c%4 (4 heads = 256 of the 1024 qkv columns / out-proj rows).
Each core computes a partial output [T, D] = ctx_heads @ Wo_slice
(+ bo on the g==0 cores); host sums the 4 partials per batch.

Numerics: matmul operands in bf16 (host pre-casts x and the weight
slices), all accumulation in fp32 PSUM; softmax denominators
accumulate in fp32; the output-projection bias is added in exact
fp32. End-to-end relative error vs the fp32 reference ~3e-3.

Device-side dataflow per core:
  1. X^T via PE-transpose (bf16, 1 cyc/row)
  2. Q^T, K^T = (Wq/Wk slice)^T @ X^T   [128 rows = 2 heads x 64dim, T]
     V natural = X @ Wv slice           [T, 4 heads x 64] + bf16 ones col
  3. per head pair / 1024-wide q-chunk / 128-wide k-tile:
       scoresT[k,q] = K_h @ Q_h^T  (K=64 contraction; the two heads of a
         pair land on array row-groups 0/64 and run concurrently)
       probsT = exp(scoresT/8) on ACT straight out of PSUM (bf16 out)
       diagonal k-tile: multiply by 0/1 causal mask on GPSIMD
       ctxT[65, q] += [V_h | ones]^T @ probsT  — row 64 accumulates the
         softmax denominator for free (fp32)
     normalize: recip(denoms) on DVE (fp32r), PE-broadcast (K=1 f32r
     matmul) across the 64 head-dim partitions, multiply during the
     PSUM->SBUF eviction; head hl=1 rows reach partitions 64..127 of
     ctxt via an SBUF->SBUF DMA (DVE is lane-locked).
  4. out = ctxT^T @ Wo slice + bo (DMA-broadcast fp32), DMA to DRAM
"""

import os
import numpy as np
import ml_dtypes

B, T, D = 2, 2048, 1024
H, HD = 16, 64
HC = 4          # heads per core
MC = HC * HD    # 256 qkv columns per core
P = 128
KO = D // P     # 8 contraction chunks for the projections
NT = T // P     # 16 token tiles
QW = 1024       # attention q-chunk width

_NC_CACHE = None


def _build_nc():
    import concourse.mybir as mybir
    import concourse.tile as tile
    from concourse import bacc
    from concourse.masks import make_identity

    dt = mybir.dt
    f32 = dt.float32
    f32r = dt.float32r
    bf16 = dt.bfloat16
    EXP = mybir.ActivationFunctionType.Exp

    nc = bacc.Bacc("TRN2", target_bir_lowering=False, debug=False, num_devices=8)

    x = nc.dram_tensor("x", [T, D], bf16, kind="ExternalInput").ap()
    wq = nc.dram_tensor("wq", [D, MC], bf16, kind="ExternalInput").ap()
    wk = nc.dram_tensor("wk", [D, MC], bf16, kind="ExternalInput").ap()
    wv = nc.dram_tensor("wv", [D, MC], bf16, kind="ExternalInput").ap()
    wo = nc.dram_tensor("wo", [MC, D], bf16, kind="ExternalInput").ap()
    bob = nc.dram_tensor("bob", [1, D], f32, kind="ExternalInput").ap()
    out = nc.dram_tensor("out", [T, D], f32, kind="ExternalOutput").ap()

    with tile.TileContext(nc) as tc:
        from contextlib import ExitStack

        with ExitStack() as ctx:
            pconst = ctx.enter_context(tc.tile_pool(name="pconst", bufs=1))
            pw = ctx.enter_context(tc.tile_pool(name="pw", bufs=1))
            pmain = ctx.enter_context(tc.tile_pool(name="pmain", bufs=1))

            # --- constants ---
            ident = pconst.tile([P, P], bf16, tag="ident")
            make_identity(nc, ident[:])
            # mask01[k, q] = 1.0 if q >= k else 0.0  (valid entries of a
            # diagonal 128x128 block in [k, q] orientation)
            mask01 = pconst.tile([P, P], bf16, tag="mask01")
            nc.gpsimd.memset(mask01[:], 1.0)
            nc.gpsimd.affine_select(
                out=mask01[:],
                in_=mask01[:],
                compare_op=mybir.AluOpType.is_ge,
                fill=0.0,
                base=0,
                pattern=[[1, P]],
                channel_multiplier=-1,
            )
            ones_f32 = pconst.tile([P, P], f32, tag="ones_f32")
            nc.vector.memset(ones_f32[:], 1.0)
            ones_r = pconst.tile([P, HD], f32r, tag="ones_r")
            nc.vector.tensor_copy(ones_r[:], ones_f32[:, 0:HD])
            ones_bf = pconst.tile([P, HD], bf16, tag="ones_bf")
            nc.vector.tensor_copy(ones_bf[:], ones_f32[:, 0:HD])
            # exact fp32 bias broadcast across partitions (DMA from DRAM
            # with a zero-stride source)
            bo_bc = pconst.tile([P, D], f32, tag="bobc")
            nc.sync.dma_start(bo_bc[:], bob[0:1, :].to_broadcast((P, D)))

            # --- weights (already bf16 from the host) ---
            wq_sb = pw.tile([P, KO, MC], bf16, tag="wq")
            wk_sb = pw.tile([P, KO, MC], bf16, tag="wk")
            wv_sb = pw.tile([P, KO, MC], bf16, tag="wv")
            wo_sb = pw.tile([P, 2, D], bf16, tag="wo")
            nc.sync.dma_start(wq_sb[:], wq.rearrange("(ko p) m -> p ko m", p=P))
            nc.sync.dma_start(wk_sb[:], wk.rearrange("(ko p) m -> p ko m", p=P))
            nc.sync.dma_start(wv_sb[:], wv.rearrange("(ko p) m -> p ko m", p=P))
            nc.sync.dma_start(wo_sb[:], wo.rearrange("(kc p) n -> p kc n", p=P))

            # --- persistent activations (all bf16) ---
            xt = pmain.tile([P, KO, T], bf16, tag="xt")            # X^T
            qt = pmain.tile([P, 2, T], bf16, tag="qt")             # Q^T
            kt_sb = pmain.tile([P, 2, T], bf16, tag="kt")          # K^T
            v_sb = pmain.tile([P, NT, HC, HD + 1], bf16, tag="v")  # V | ones
            ctxt = pmain.tile([P, 2, T], bf16, tag="ctxt")         # ctx^T

            # ================= phase 1: X^T =================
            with (
                tc.tile_pool(name="px", bufs=3) as px,
                tc.tile_pool(name="ps1", bufs=4, space="PSUM") as ps1,
            ):
                for tt in range(NT):
                    xs = px.tile([P, D], bf16, tag="xs")
                    nc.sync.dma_start(xs[:], x[P * tt : P * (tt + 1), :])
                    pst = ps1.tile([P, KO * P], bf16, tag="pst")
                    for ko in range(KO):
                        nc.tensor.transpose(
                            pst[:, P * ko : P * (ko + 1)],
                            xs[:, P * ko : P * (ko + 1)],
                            ident[:],
                        )
                    nc.vector.tensor_copy(
                        xt[:, :, P * tt : P * (tt + 1)],
                        pst[:].rearrange("p (ko q) -> p ko q", ko=KO),
                    )

            # ================= phase 2: projections =================
            with (
                tc.tile_pool(name="ps2", bufs=1, space="PSUM") as ps2,
                tc.tile_pool(name="psv", bufs=4, space="PSUM") as psv,
            ):
                for w_sb, dst in ((wq_sb, qt), (wk_sb, kt_sb)):
                    for mc in range(2):
                        ps = ps2.tile([P, T], f32, tag="proj", name="proj")
                        for ko in range(KO):
                            for nq in range(4):
                                nc.tensor.matmul(
                                    ps[:, 512 * nq : 512 * (nq + 1)],
                                    lhsT=w_sb[:, ko, P * mc : P * (mc + 1)],
                                    rhs=xt[:, ko, 512 * nq : 512 * (nq + 1)],
                                    start=(ko == 0),
                                    stop=(ko == KO - 1),
                                )
                        nc.vector.tensor_copy(dst[:, mc, :], ps[:])

                nc.vector.tensor_copy(
                    v_sb[:, :, :, HD],
                    ones_f32[:, 0 : NT * HC].rearrange("p (t h) -> p t h", t=NT),
                )
                for tt in range(NT):
                    pv = psv.tile([P, MC], f32, tag="v", name="pv")
                    for ko in range(KO):
                        nc.tensor.matmul(
                            pv[:],
                            lhsT=xt[:, ko, P * tt : P * (tt + 1)],
                            rhs=wv_sb[:, ko, :],
                            start=(ko == 0),
                            stop=(ko == KO - 1),
                        )
                    nc.vector.tensor_copy(
                        v_sb[:, tt, :, 0:HD],
                        pv[:].rearrange("p (h d) -> p h d", h=HC),
                    )

            # ================= phase 3: attention =================
            with (
                tc.tile_pool(name="psc", bufs=2, space="PSUM") as psc,
                tc.tile_pool(name="pctx", bufs=2, space="PSUM") as pctx,
                tc.tile_pool(name="pprob", bufs=4) as pprob,
                tc.tile_pool(name="pnorm", bufs=1) as pnorm,
            ):
                for mc in range(2):
                    for qn in range(T // QW):
                        qbase = QW * qn
                        nkt = (qbase + QW) // P
                        ctx2 = [
                            pctx.tile([HD + 1, QW], f32, tag="ctx", name="ctx")
                            for _ in range(2)
                        ]
                        for kti in range(nkt):
                            qstart = max(qbase, P * kti)
                            qlen = qbase + QW - qstart
                            rel = qstart - qbase
                            diag = P * kti >= qbase
                            for hl in range(2):
                                head = 2 * mc + hl
                                hp = slice(HD * hl, HD * (hl + 1))
                                sc = psc.tile([P, QW], f32, tag="sc", name="sc")
                                for o in range(0, qlen, 512):
                                    l = min(512, qlen - o)
                                    nc.tensor.matmul(
                                        sc[:, o : o + l],
                                        lhsT=kt_sb[hp, mc, P * kti : P * (kti + 1)],
                                        rhs=qt[hp, mc, qstart + o : qstart + o + l],
                                        start=True,
                                        stop=True,
                                    )
                                probs = pprob.tile(
                                    [P, QW], bf16, tag="probs", name="probs"
                                )
                                nc.scalar.activation(
                                    probs[:, 0:qlen], sc[:, 0:qlen], EXP, scale=0.125
                                )
                                if diag:
                                    nc.gpsimd.tensor_mul(
                                        probs[:, 0:P], probs[:, 0:P], mask01[:]
                                    )
                                # accumulate ctx^T (+denominator row) over k-tiles;
                                # chunks split on PSUM bank boundaries
                                edges = [rel] + [
                                    e for e in range(512, QW + 1, 512) if e > rel
                                ]
                                for a, b_ in zip(edges[:-1], edges[1:]):
                                    nc.tensor.matmul(
                                        ctx2[hl][:, a:b_],
                                        lhsT=v_sb[:, kti, head, :],
                                        rhs=probs[:, a - rel : b_ - rel],
                                        start=(kti == 0),
                                        stop=(kti == nkt - 1),
                                        skip_group_check=True,
                                    )
                        # normalize: 1/denom (row 64 of ctx2), PE-broadcast
                        # to partitions 0..63, multiply during PSUM eviction.
                        # DVE is lane-locked, so head hl=1's rows are moved to
                        # partitions 64..127 of ctxt by an SBUF->SBUF DMA.
                        for hl in range(2):
                            rinv = pnorm.tile(
                                [HD + 1, QW], f32r, tag="rinv", name="rinv"
                            )
                            with nc.allow_low_precision(reason="fp32r"):
                                nc.vector.reciprocal(
                                    rinv[HD : HD + 1, :], ctx2[hl][HD : HD + 1, :]
                                )
                            rps = psc.tile([P, QW], f32, tag="sc", name="rps")
                            for o in range(0, QW, 512):
                                nc.tensor.matmul(
                                    rps[0:HD, o : o + 512],
                                    lhsT=ones_r[HD : HD + 1, :],
                                    rhs=rinv[HD : HD + 1, o : o + 512],
                                    start=True,
                                    stop=True,
                                )
                            rsb = pnorm.tile([HD, QW], f32, tag="rsb", name="rsb")
                            nc.vector.tensor_copy(rsb[:], rps[0:HD, :])
                            if hl == 0:
                                nc.vector.tensor_mul(
                                    ctxt[0:HD, mc, qbase : qbase + QW],
                                    ctx2[0][0:HD, :],
                                    rsb[:],
                                )
                            else:
                                stage = pnorm.tile(
                                    [HD, QW], bf16, tag="stage", name="stage"
                                )
                                nc.vector.tensor_mul(
                                    stage[:], ctx2[1][0:HD, :], rsb[:]
                                )
                                nc.sync.dma_start(
                                    ctxt[HD:P, mc, qbase : qbase + QW], stage[:]
                                )

            # ================= phase 4: output projection =================
            with (
                tc.tile_pool(name="ps4", bufs=3, space="PSUM") as ps4,
                tc.tile_pool(name="po", bufs=3) as po_pool,
            ):
                for tt in range(NT):
                    pp = ps4.tile([P, D], f32, tag="po", name="pp")
                    for mc in range(2):
                        for nn in range(2):
                            nc.tensor.matmul(
                                pp[:, 512 * nn : 512 * (nn + 1)],
                                lhsT=ctxt[:, mc, P * tt : P * (tt + 1)],
                                rhs=wo_sb[:, mc, 512 * nn : 512 * (nn + 1)],
                                start=(mc == 0),
                                stop=(mc == 1),
                            )
                    o_sb = po_pool.tile([P, D], f32, tag="o", name="o_sb")
                    nc.vector.tensor_add(o_sb[:], pp[:], bo_bc[:])
                    nc.sync.dma_start(out[P * tt : P * (tt + 1), :], o_sb[:])

    nc.compile()
    return nc


def get_nc():
    global _NC_CACHE
    if _NC_CACHE is None:
        _NC_CACHE = _build_nc()
    return _NC_CACHE


def make_in_maps(x, Wq, Wk, Wv, Wo, bo):
    bf = ml_dtypes.bfloat16
    x = np.asarray(x, dtype=np.float32).astype(bf)
    Wq = np.asarray(Wq, dtype=np.float32).astype(bf)
    Wk = np.asarray(Wk, dtype=np.float32).astype(bf)
    Wv = np.asarray(Wv, dtype=np.float32).astype(bf)
    Wo = np.asarray(Wo, dtype=np.float32).astype(bf)
    bo = np.asarray(bo, dtype=np.float32)
    zeros_bo = np.zeros((1, D), np.float32)
    in_maps = []
    for c in range(8):
        b, g = divmod(c, 4)
        sl = slice(MC * g, MC * (g + 1))
        in_maps.append(
            {
                "x": np.ascontiguousarray(x[b]),
                "wq": np.ascontiguousarray(Wq[:, sl]),
                "wk": np.ascontiguousarray(Wk[:, sl]),
                "wv": np.ascontiguousarray(Wv[:, sl]),
                "wo": np.ascontiguousarray(Wo[sl, :]),
                "bob": bo.reshape(1, D) if g == 0 else zeros_bo,
            }
        )
    return in_maps


def _install_profile_hook():
    """Register the axon NTFF profiling hook (the image's antenv lacks
    axon_hooks, so the boot-time registration degraded silently)."""
    import sys
    import types

    if "antenv.axon_hooks" not in sys.modules:
        m = types.ModuleType("antenv.axon_hooks")
        m._hook = None
        m.set_axon_ntff_profile_hook = lambda h: setattr(m, "_hook", h)
        m.get_axon_ntff_profile_hook = lambda: m._hook
        sys.modules["antenv.axon_hooks"] = m
        import antenv

        antenv.axon_hooks = m
    if "/root/.axon_site" not in sys.path:
        sys.path.append("/root/.axon_site")
    from trn_agent_boot.trn_boot import _ntff_profile_via_ctypes

    sys.modules["antenv.axon_hooks"].set_axon_ntff_profile_hook(
        _ntff_profile_via_ctypes("/opt/axon/libaxon_pjrt.so")
    )


def kernel_with_results(x, Wq, Wk, Wv, Wo, bo, trace=False):
    from concourse.bass_utils import run_bass_kernel_spmd

    if trace:
        _install_profile_hook()
    nc = get_nc()
    in_maps = make_in_maps(x, Wq, Wk, Wv, Wo, bo)
    res = run_bass_kernel_spmd(nc, in_maps, core_ids=list(range(8)), trace=trace)
    parts = [r["out"] for r in res.results]
    full = np.stack(
        [
            parts[0] + parts[1] + parts[2] + parts[3],
            parts[4] + parts[5] + parts[6] + parts[7],
        ]
    )
    return full, res


def kernel(x, Wq, Wk, Wv, Wo, bo):
    full, _ = kernel_with_results(
        x, Wq, Wk, Wv, Wo, bo, trace=bool(os.environ.get("KERNEL_TRACE"))
    )
    return full
